# revision 20
# baseline (speedup 1.0000x reference)
"""Bi-directional minGRU kernel for Trainium2 (8 NeuronCores, Bass/Tile).

Strategy
--------
Data-parallel over batch: B=256 examples sharded 32 per core. Per example all
tensors live in feature-major layout [feature->partition, time->free]; the
minGRU recurrence is a hardware `tensor_tensor_scan` along the free (time)
axis: rows 0-63 carry the forward direction in normal time order, rows 64-127
carry the backward direction in *reversed* time order (reversal is free on
matmul rhs operands via negative-stride views).

vs the v1 baseline:
  * all matmuls run as float32r (1 cycle/row at N>=256 instead of fp32's 4)
  * the reversed-mask broadcast comes from a host-precomputed reversed row in
    DRAM (contiguous descriptors) instead of an on-chip stride -1 DMA that
    exploded into per-element descriptors; the broadcast tile is bf16
  * the masked fixup E = mask * (pre - final) is ONE scalar_tensor_tensor
  * elementwise work is balanced: Act {sigmoid, tanh, te-relu}, DVE {B=z*th,
    scan, head-relu}, Pool/GpSimd {A=1-z, E}
  * the per-example stage chain is software-pipelined across examples with
    per-engine issue offsets so no engine queue head ever waits on same-
    iteration work from another engine
  * head2 preds accumulate into one PSUM bank per example (partition rows
    0/32/64/96) and DMA straight from PSUM to DRAM

Host-side (numpy, fp64 then cast to fp32) the linear chains are fused:
    gz_d = (wz@proj[:, :3]) @ x3 + (wz@proj[:, 3:]@te_w2) @ relu_te1 + bias
so the time encoder's second layer, the input projections, and the gate
weights collapse into single [67 -> 64] matmuls, and the head's te branch
collapses into W1t2 = gh_w1[:,128:] @ te_w2. The masked-position fixup
h_apply = m*pre + (1-m)*final commutes with the head matmul, so the kernel
computes E = m*(pre - final), one [128,1] matmul for W@final folded into the
head bias column, and never materializes h_apply.
"""
import os
import sys

for _p in ("/opt/trn_rl_repo", "/root/.axon_site/_ro/trn_rl_repo"):
    if os.path.isdir(_p) and _p not in sys.path:
        sys.path.insert(0, _p)

import numpy as np
from contextlib import ExitStack

import concourse.bacc as bacc
import concourse.tile as tile
import concourse.mybir as mybir
from concourse.bass_utils import run_bass_kernel_spmd

F32 = mybir.dt.float32
F32R = mybir.dt.float32r
BF16 = mybir.dt.bfloat16
AF = mybir.ActivationFunctionType
OP = mybir.AluOpType

B, L, H, TE = 256, 2048, 64, 64
NCORES = 8
BS = B // NCORES          # examples per core
NW = 709                  # packed weight columns (see _pack_weights)

# weight column layout inside the packed [128, NW] tile
_C_W1FB = 0               # lhsT of [W1f | W1b], [128, 128]
_C_W1T2 = 128             # lhsT of W1t2, rows 0-63, [64, 128]
_C_ZF = 256               # gate lhsTs, rows 0-66, [67, 64] each
_C_HF = 320
_C_ZB = 384
_C_HB = 448
_C_A1 = 512               # te_w1 lhsT, row 96 only, [1, 64]
_C_W2T = 576              # gh_w2^T, [128, 1]
_C_ZBIAS = 577            # gate z bias column [128, 1]
_C_HBIAS = 578            # gate h bias column
_C_HEADB = 579            # head bias column (gh_b1 + W1t@te_b2)
_C_B1 = 580               # te bias, rows 0-63
_C_W1B0 = 581             # lhsT of W1b zero-padded to K=128 (rows 0-63 zero)

_cache = {}


def _pack_weights(inp):
    """Fuse the linear chains (fp64) and pack every lhsT into one [128, NW]
    fp32 array whose column slices are the matmul stationary operands."""
    g = {k: np.asarray(v, np.float64) for k, v in inp.items()}
    wts = np.zeros((128, NW), np.float64)

    def fuse(proj_w, proj_b, wz, bz, wh, bh):
        P3 = proj_w[:, :3]
        Pte_te2 = proj_w[:, 3:] @ g["te_w2"]
        pbias = proj_w[:, 3:] @ g["te_b2"] + proj_b
        # x3 rows on the device are ordered [mask, x1, x2]
        reord = np.stack([P3[:, 2], P3[:, 0], P3[:, 1]], axis=1)
        return (
            np.concatenate([wz @ Pte_te2, wz @ reord], axis=1),  # (64, 67)
            wz @ pbias + bz,
            np.concatenate([wh @ Pte_te2, wh @ reord], axis=1),
            wh @ pbias + bh,
        )

    Zf, zbf, Hf, hbf = fuse(g["fproj_w"], g["fproj_b"], g["fwz"], g["fbz"],
                            g["fwh"], g["fbh"])
    Zb, zbb, Hb, hbb = fuse(g["bproj_w"], g["bproj_b"], g["bwz"], g["bbz"],
                            g["bwh"], g["bbh"])
    # gate lhsT: [K=67 rows: 0-63 r, 64 mask, 65 x1, 66 x2][M=64]
    wts[0:67, _C_ZF:_C_ZF + 64] = Zf.T
    wts[0:67, _C_HF:_C_HF + 64] = Hf.T
    wts[0:67, _C_ZB:_C_ZB + 64] = Zb.T
    wts[0:67, _C_HB:_C_HB + 64] = Hb.T
    wts[0:64, _C_ZBIAS] = zbf
    wts[64:128, _C_ZBIAS] = zbb
    wts[0:64, _C_HBIAS] = hbf
    wts[64:128, _C_HBIAS] = hbb
    # head
    W1f = g["gh_w1"][:, :64]
    W1b = g["gh_w1"][:, 64:128]
    W1t = g["gh_w1"][:, 128:192]
    W1fb = np.concatenate([W1f, W1b], axis=1)          # (128, 128)
    wts[0:128, _C_W1FB:_C_W1FB + 128] = W1fb.T
    wts[0:64, _C_W1T2:_C_W1T2 + 128] = (W1t @ g["te_w2"]).T
    wts[0:128, _C_HEADB] = g["gh_b1"] + W1t @ g["te_b2"]
    # te first layer: lhsT row 96 (rhs = t row at partition 96)
    wts[96, _C_A1:_C_A1 + 64] = g["te_w1"][:, 0]
    wts[0:64, _C_B1] = g["te_b1"]
    # head2
    wts[0:128, _C_W2T] = g["gh_w2"][0]
    # W1b lhsT zero-padded to K=128 (rows 0-63 zero): multiplies the reversed
    # full [128]-row E view; the top rows hit zero weights.
    wts[64:128, _C_W1B0:_C_W1B0 + 128] = W1b.T
    return np.ascontiguousarray(wts, np.float32), np.float32(g["gh_b2"][0])


def _build_program():
    """Build + compile the 8-core SPMD Bass program once.

    Software pipeline: iteration i issues (work item -> stage offset)
      DMA   : xr(i), mb(i), out(i-7)
      PE    : gates(i-2), te(i-1), sbb-mm(i-5), head1(i-6), head2(i-7)
      Act   : gate-acts(i-2), te-act(i-1), sbb-act(i-5), preds-copy(i-7)
      DVE   : scan(i-4), E(i-5), head-relu(i-6)
      Pool  : A(i-3), B(i-3)
    """
    nc = bacc.Bacc("TRN2", num_devices=NCORES, debug=False)
    wts_d = nc.dram_tensor("wts", [128, NW], F32R, kind="ExternalInput")
    wtsb_d = nc.dram_tensor("wtsb", [128, NW], BF16, kind="ExternalInput")
    inx_d = nc.dram_tensor("inx", [BS, 4, L], BF16, kind="ExternalInput")
    mrw_d = nc.dram_tensor("mrw", [BS, 2, L], BF16, kind="ExternalInput")
    out_d = nc.dram_tensor("out", [BS, 4, L // 4], F32, kind="ExternalOutput")

    r32 = lambda ap: ap.bitcast(F32R)

    with tile.TileContext(nc) as tc, ExitStack() as ctx:
        wpool = ctx.enter_context(tc.tile_pool(name="w", bufs=1))
        p_xr = ctx.enter_context(tc.tile_pool(name="xr", bufs=7))
        p_mb = ctx.enter_context(tc.tile_pool(name="mb", bufs=6))
        p_z = ctx.enter_context(tc.tile_pool(name="z", bufs=2))
        p_th = ctx.enter_context(tc.tile_pool(name="th", bufs=2))
        p_a = ctx.enter_context(tc.tile_pool(name="a", bufs=2))
        p_b = ctx.enter_context(tc.tile_pool(name="b", bufs=2))
        p_hs = ctx.enter_context(tc.tile_pool(name="hs", bufs=2))
        p_e = ctx.enter_context(tc.tile_pool(name="e", bufs=2))
        p_hid = ctx.enter_context(tc.tile_pool(name="hid", bufs=2))
        p_sbb = ctx.enter_context(tc.tile_pool(name="sbb", bufs=3))
        # PSUM: pool A [128,1024] (2 banks) x3 = 6 banks for te/gates/head1,
        # pool B [128,512] (1 bank) x2 = 2 banks for sbb + head2 preds
        ps_a = ctx.enter_context(tc.tile_pool(name="psa", bufs=3, space="PSUM"))
        ps_b = ctx.enter_context(tc.tile_pool(name="psb", bufs=2, space="PSUM"))

        wtsr = wpool.tile([128, NW], F32R, tag="wts")
        nc.sync.dma_start(wtsr[:], wts_d.ap()[:])
        wts = wtsr.bitcast(F32)
        wtsb = wpool.tile([128, NW], BF16, tag="wtsb")
        nc.sync.dma_start(wtsb[:], wtsb_d.ap()[:])
        inx = inx_d.ap()
        mrw = mrw_d.ap()

        # per-example live tiles, indexed by example id
        T = {}

        def dma_in(e):
            xr = p_xr.tile([128, L], BF16, tag="xr", name=f"xr{e}")
            # rows 64-66 <- [mask, x1, x2]; row 96 <- t
            nc.sync.dma_start(xr[64:67, :], inx[e, 0:3, :])
            nc.sync.dma_start(xr[96:97, :], inx[e, 3:4, :])
            mb = p_mb.tile([128, L], BF16, tag="mb", name=f"mb{e}")
            # rows 0-63 mask (normal), rows 64-127 mask reversed
            nc.sync.dma_start(mb[0:64, :], mrw[e, 0:1, :].broadcast_to((64, L)))
            nc.sync.dma_start(mb[64:128, :],
                              mrw[e, 1:2, :].broadcast_to((64, L)))
            T[e] = {"xr": xr, "mb": mb}

        def pe_te(e):
            t = T[e]
            t["pste"] = []
            for c in range(2):
                pst = ps_a.tile([128, 1024], F32, tag="psa", name=f"te{e}_{c}")
                for s in range(2):
                    cs = slice((2 * c + s) * 512, (2 * c + s + 1) * 512)
                    nc.tensor.matmul(pst[0:64, s * 512:(s + 1) * 512],
                                     wtsb[96:97, _C_A1:_C_A1 + 64],
                                     t["xr"][96:97, cs],
                                     start=True, stop=True,
                                     tile_position=(96, 0))
                t["pste"].append(pst)

        def act_te(e):
            t = T[e]
            for c in range(2):
                cs = slice(c * 1024, (c + 1) * 1024)
                nc.scalar.activation(t["xr"][0:64, cs], t["pste"][c][0:64, :],
                                     AF.Relu, bias=wts[0:64, _C_B1:_C_B1 + 1])
            del t["pste"]

        def pe_gates(e):
            t = T[e]
            xr = t["xr"]
            xrev = xr[0:67, ::-1]
            t["psg"] = []
            for (cf, cb) in ((_C_ZF, _C_ZB), (_C_HF, _C_HB)):
                for c in range(2):
                    psg = ps_a.tile([128, 1024], F32, tag="psa",
                                    name=f"g{e}_{cf}_{c}")
                    for s in range(2):
                        cs = slice((2 * c + s) * 512, (2 * c + s + 1) * 512)
                        ds = slice(s * 512, (s + 1) * 512)
                        nc.tensor.matmul(psg[0:64, ds],
                                         wtsb[0:67, cf:cf + 64],
                                         xr[0:67, cs],
                                         start=True, stop=True,
                                         tile_position=(0, 0))
                        nc.tensor.matmul(psg[64:128, ds],
                                         wtsb[0:67, cb:cb + 64],
                                         xrev[:, cs],
                                         start=True, stop=True,
                                         tile_position=(0, 64))
                    t["psg"].append(psg)

        def act_gates(e):
            t = T[e]
            Z = p_z.tile([128, L], F32, tag="z", name=f"z{e}")
            TH = p_th.tile([128, L], F32, tag="th", name=f"th{e}")
            for i, (dst, bias_c, fn) in enumerate(
                    ((Z, _C_ZBIAS, AF.Sigmoid), (TH, _C_HBIAS, AF.Tanh))):
                for c in range(2):
                    cs = slice(c * 1024, (c + 1) * 1024)
                    nc.scalar.activation(dst[:, cs], t["psg"][2 * i + c][:],
                                         fn, bias=wts[:, bias_c:bias_c + 1])
            del t["psg"]
            t["Z"], t["TH"] = Z, TH

        def pool_ab(e):
            # A = 1 - z and B = z * h~ both on GpSimd (the only two movable
            # SBUF-only elementwise ops; DVE is full with scan/E/head-relu)
            t = T[e]
            A = p_a.tile([128, L], F32, tag="a", name=f"a{e}")
            nc.gpsimd.tensor_scalar(A[:], t["Z"][:], -1.0, 1.0,
                                    OP.mult, OP.add)
            Bt = p_b.tile([128, L], F32, tag="b", name=f"b{e}")
            nc.gpsimd.tensor_tensor(Bt[:], t["Z"][:], t["TH"][:], OP.mult)
            t["A"], t["Bt"] = A, Bt
            del t["Z"], t["TH"]

        def dve_scan(e):
            t = T[e]
            Hs = p_hs.tile([128, L + 1], F32, tag="hs", name=f"hs{e}")
            nc.vector.memset(Hs[:, 0:1], 0.0)
            nc.vector.tensor_tensor_scan(Hs[:, 1:L + 1], t["A"][:], t["Bt"][:],
                                         0.0, OP.mult, OP.add)
            t["Hs"] = Hs
            del t["A"], t["Bt"]

        def dve_e(e):
            t = T[e]
            E = p_e.tile([128, L], BF16, tag="e", name=f"ee{e}")
            # E = (pre - final) * mask   (one fused scalar_tensor_tensor)
            nc.vector.scalar_tensor_tensor(
                E[:], t["Hs"][:, 0:L], t["Hs"][:, L - 1:L],
                t["mb"][:], OP.subtract, OP.mult)
            t["E"] = E
            del t["mb"]

        def pe_sbb(e):
            t = T[e]
            psv = ps_b.tile([128, 512], F32, tag="psb", name=f"sv{e}")
            nc.tensor.matmul(psv[:, 0:1], wts[:, _C_W1FB:_C_W1FB + 128],
                             t["Hs"][:, L - 1:L], start=True, stop=True,
                             tile_position=(0, 0))
            sbb = p_sbb.tile([128, 1], F32, tag="sbb", name=f"sb{e}")
            nc.scalar.activation(sbb[:], psv[:, 0:1], AF.Identity,
                                 bias=wts[:, _C_HEADB:_C_HEADB + 1])
            t["sbb"] = sbb
            t["psv"] = psv

        def act_sbb(e):
            pass  # folded into pe_sbb issue order (tiny)

        def pe_head1(e):
            t = T[e]
            E = t["E"]
            Erev = E[0:128, ::-1]
            t["psS"] = []
            for c in range(2):
                psS = ps_a.tile([128, 1024], F32, tag="psa", name=f"h1{e}_{c}")
                for s in range(2):
                    cs = slice((2 * c + s) * 512, (2 * c + s + 1) * 512)
                    ds = slice(s * 512, (s + 1) * 512)
                    nc.tensor.matmul(psS[:, ds],
                                     wtsb[0:64, _C_W1FB:_C_W1FB + 128],
                                     E[0:64, cs], start=True, stop=False,
                                     tile_position=(0, 0))
                    nc.tensor.matmul(psS[:, ds],
                                     wtsb[0:128, _C_W1B0:_C_W1B0 + 128],
                                     Erev[:, cs], start=False, stop=False,
                                     tile_position=(0, 0))
                    nc.tensor.matmul(psS[:, ds],
                                     wtsb[0:64, _C_W1T2:_C_W1T2 + 128],
                                     t["xr"][0:64, cs], start=False,
                                     stop=True, tile_position=(0, 0))
                t["psS"].append(psS)

        def dve_hrelu(e):
            t = T[e]
            hid = p_hid.tile([128, L], BF16, tag="hid", name=f"hd{e}")
            for c in range(2):
                cs = slice(c * 1024, (c + 1) * 1024)
                nc.vector.tensor_scalar(hid[:, cs], t["psS"][c][:],
                                        t["sbb"][:], 0.0, OP.add, OP.max)
            t["hid"] = hid
            del t["psS"], t["E"], t["xr"], t["sbb"]

        def pe_head2(e):
            t = T[e]
            psP = ps_b.tile([128, 512], F32, tag="psb", name=f"pp{e}")
            for c in range(4):
                cs = slice(c * 512, (c + 1) * 512)
                nc.tensor.matmul(psP[32 * c:32 * c + 1, :],
                                 wtsb[:, _C_W2T:_C_W2T + 1],
                                 t["hid"][:, cs], start=True, stop=True,
                                 tile_position=(0, 32 * c))
            t["psP"] = psP
            del t["hid"]

        def dma_out(e):
            t = T[e]
            pg = p_sbb.tile([128, 512], F32, tag="pg", name=f"pg{e}")
            # copy the contiguous 0..96 partition range (rows between the 4
            # preds rows are dead); the DMA then strides over them
            nc.scalar.activation(pg[0:97, :], t["psP"][0:97, :], AF.Copy)
            nc.sync.dma_start(out_d.ap()[e], pg[0:128:32, :])
            del t["psP"], t["Hs"]
            del T[e]

        # Issue order is per-engine program order. Per iteration i:
        #   PE  : gates(i-2), te(i-1), sbb(i-5), head1(i-6), head2(i-7)
        #   Act : gate-acts(i-2), te-act(i-1), sbb-act(i-5), pg(i-7)
        #   DVE : scan(i-4), E(i-5), head-relu(i-6)
        #   Pool: A(i-3), B(i-3)
        #   DMA : in(i), out(i-7)
        # Every dependency is >= 1 iteration old at issue time except
        # producer->consumer pairs that trail each other within an iteration
        # (matmul chunks -> acts, head1 psum -> head-relu).
        NSTAGE = 8
        for i in range(BS + NSTAGE):
            if i < BS:
                dma_in(i)
            if 0 <= i - 2 < BS:
                pe_gates(i - 2)
                act_gates(i - 2)
            if 0 <= i - 1 < BS:
                pe_te(i - 1)
                act_te(i - 1)
            if 0 <= i - 3 < BS:
                pool_ab(i - 3)
            if 0 <= i - 4 < BS:
                dve_scan(i - 4)
            if 0 <= i - 5 < BS:
                pe_sbb(i - 5)
                dve_e(i - 5)
            if 0 <= i - 6 < BS:
                pe_head1(i - 6)
                dve_hrelu(i - 6)
            if 0 <= i - 7 < BS:
                pe_head2(i - 7)
                dma_out(i - 7)

    nc.compile()
    return nc


def prep_in_maps(x, t, mask_token, weights):
    """Host-side input prep: returns (in_maps for the 8 cores, output bias)."""
    wts, b2 = _pack_weights(weights)
    x = np.asarray(x, np.float32)
    t = np.asarray(t, np.float32)
    tok = np.asarray(mask_token, np.float32)
    xT = np.swapaxes(x, 1, 2)                    # (B, 3, L)
    mask = xT[:, 2:3, :]
    x12 = np.where(mask == 0, tok.reshape(1, 2, 1), xT[:, 0:2, :])
    tn = np.swapaxes(t, 1, 2)                    # (B, 1, L)
    import ml_dtypes
    inx = np.ascontiguousarray(
        np.concatenate([mask, x12, tn], axis=1)
    ).astype(ml_dtypes.bfloat16)                 # (B, 4, L) bf16
    mrw = np.ascontiguousarray(
        np.concatenate([mask, mask[:, :, ::-1]], axis=1)
    ).astype(ml_dtypes.bfloat16)                 # (B, 2, L) bf16
    wtsb = wts.astype(ml_dtypes.bfloat16)
    in_maps = [
        {"wts": wts, "wtsb": wtsb, "inx": inx[c * BS:(c + 1) * BS],
         "mrw": mrw[c * BS:(c + 1) * BS]}
        for c in range(NCORES)
    ]
    return in_maps, b2


def kernel(x, t, mask_token,
           te_w1, te_b1, te_w2, te_b2,
           fproj_w, fproj_b, bproj_w, bproj_b,
           fwz, fbz, fwh, fbh,
           bwz, bbz, bwh, bbh,
           gh_w1, gh_b1, gh_w2, gh_b2):
    inp = dict(te_w1=te_w1, te_b1=te_b1, te_w2=te_w2, te_b2=te_b2,
               fproj_w=fproj_w, fproj_b=fproj_b, bproj_w=bproj_w,
               bproj_b=bproj_b, fwz=fwz, fbz=fbz, fwh=fwh, fbh=fbh,
               bwz=bwz, bbz=bbz, bwh=bwh, bbh=bbh,
               gh_w1=gh_w1, gh_b1=gh_b1, gh_w2=gh_w2, gh_b2=gh_b2)
    in_maps, b2 = prep_in_maps(x, t, mask_token, inp)

    if "nc" not in _cache:
        _cache["nc"] = _build_program()
    nc = _cache["nc"]

    res = run_bass_kernel_spmd(nc, in_maps, core_ids=list(range(NCORES)))
    out = np.concatenate([res.results[c]["out"].reshape(BS, L)
                          for c in range(NCORES)], axis=0)
    return (out + b2).reshape(B, L, 1).astype(np.float32)
